# revision 19
# baseline (speedup 1.0000x reference)
"""Trainium2 Bass kernel for nn_BinaryLayer (logic-gate network).

Computes: out[b, o] = OR_t AND_a x_in[b, weights[o, t, a]]
where x_in = [const_true | (x != 0) | ~(x != 0)]  (width 1 + 2*784 = 1569),
plus an or-mask: an (o, t) gate whose 16 indices are all 0 is forced False.

Strategy (8 NeuronCores, tensor-parallel over OUT: 128 outs per core):
  The AND over a gate's 16 terms equals (sum of its selected bits == 16).
  Each selected bit is const-1, x[f], or 1-x[f], so the sum is affine in x:
      S[b, o, t] = sum_r M[r, (o,t)] * xr[b, r]
  over 785 rows (row 0 = const-1 carrying the gate's base count, rows
  1..784 = features), with integer M (<=16 nonzeros per column, |M| <= 16;
  masked gates get an all-zero column so S = 0 < 16).
  Since S <= 16 always,  OR_t (S_t == 16)  ==  (max_t S_t == 16).

  Per core pipeline (per 512-batch chunk):
    1. PE: 32 S-tiles [128 o, 512 b] (tile = one or-term t per output), as
       fp8(e4m3) DoubleRow matmuls over 1024 padded rows (4 passes of 256;
       exact: all values are small integers; accumulation is fp32 PSUM).
       Pass-4 skipping: rows are permuted so pass 4 holds only the 17
       least-referenced features, and each output's gates are ordered so
       gates referencing those rows sit in the last tiles. Tiles whose
       columns have all-zero pass-4 rows skip the 4th matmul (exact).
       The tile count needing pass 4 (n4) is weight-dependent, so the
       program is compiled per n4 (cached).
    2. DVE: pairwise max of adjacent S-tile pairs (fp32 PSUM -> bf16 SBUF,
       exact for |S| <= 16), then a running bf16 max (2x DVE mode).
    3. ACT: out u8 = relu(max - 15) in {0,1}; DMA out [128 o, 1024 b].
"""

import numpy as np

B, F = 1024, 784
OUT, OR_T, AND_T = 1024, 32, 16
N_CORES = 8
O_LOC = OUT // N_CORES  # 128 outs per core
OT_LOC = O_LOC * OR_T  # 4096 (o,t) columns per core
NK = 4  # max DoubleRow k-passes over 1024 padded rows (256 each)
NROW = 1 + F  # 785 live contraction rows (const/base row + features)
NTAIL = NROW - 768  # 17 rows assigned to pass 4
NB = 2  # batch chunks
BCH = B // NB  # 512

_cache = {}


def _build(reps=1, loop=False, inner=1, n4=OR_T):
    import concourse.mybir as mybir
    import concourse.tile as tile
    from concourse.bacc import Bacc

    f32 = mybir.dt.float32
    bf16 = mybir.dt.bfloat16
    u8 = mybir.dt.uint8
    f8 = mybir.dt.float8e4
    Act = mybir.ActivationFunctionType
    Alu = mybir.AluOpType
    DR = mybir.MatmulPerfMode.DoubleRow

    nc = Bacc("TRN2", target_bir_lowering=False, debug=False, num_devices=N_CORES)
    xq_t = nc.dram_tensor("xq", [128, NK, 2, B], u8, kind="ExternalInput")
    mq_t = nc.dram_tensor("mq", [128, NK, 2, OT_LOC], u8, kind="ExternalInput")
    out_t = nc.dram_tensor("out", [128, B], u8, kind="ExternalOutput")

    with tile.TileContext(nc) as tc:
        with (
            tc.tile_pool(name="main", bufs=1) as pool,
            tc.tile_pool(name="tmpp", bufs=4) as tpool,
            tc.tile_pool(name="runp", bufs=2) as rpool,
            tc.tile_pool(name="outp", bufs=2) as opool,
            tc.tile_pool(name="pp1", bufs=8, space="PSUM") as pp1,
        ):
            xq = pool.tile([128, NK, 2, B], u8)
            mq = pool.tile([128, NK, 2, OT_LOC], u8)
            # split DMAs: xq per k-pass, mq per ot-range (512 cols = 4 tiles)
            # so the first matmuls can start before all input lands.
            for k in range(NK):
                nc.sync.dma_start(xq[:, k], xq_t.ap()[:, k])
            for r in range(8):
                nc.sync.dma_start(
                    mq[:, :, :, 512 * r : 512 * (r + 1)],
                    mq_t.ap()[:, :, :, 512 * r : 512 * (r + 1)],
                )
            xqf = xq[:].bitcast(f8)
            mqf = mq[:].bitcast(f8)
            bias15 = pool.tile([128, 1], f32)
            nc.vector.memset(bias15[:], -15.0)

            def s_tile(t, bc):
                nk_t = NK if t >= OR_T - n4 else NK - 1
                ps1 = pp1.tile([128, BCH], f32, tag="ps1")
                for k in range(nk_t):
                    nc.tensor.matmul(
                        out=ps1[:],
                        lhsT=mqf[:, k, :, 128 * t : 128 * (t + 1)],
                        rhs=xqf[:, k, :, BCH * bc : BCH * (bc + 1)],
                        start=(k == 0),
                        stop=(k == nk_t - 1),
                        perf_mode=DR,
                        skip_group_check=True,
                    )
                return ps1

            def body(_i=None):
                for bc in range(NB):
                    # ACT evacuates each S-tile psum -> bf16 sbuf (exact for
                    # |S| <= 16); DVE runs the max chain on bf16 in SBUF.
                    run = rpool.tile([128, BCH], bf16, tag="run")
                    for t in range(OR_T):
                        ps1 = s_tile(t, bc)
                        if t == 0:
                            nc.scalar.copy(run[:], ps1[:])
                        else:
                            ev = tpool.tile([128, BCH], bf16, tag="ev")
                            # every 4th evac on DVE to keep ACT well under
                            # the PE rate (avoids psum-bank backpressure)
                            if t % 4 == 3:
                                nc.vector.tensor_copy(out=ev[:], in_=ps1[:])
                            else:
                                nc.scalar.copy(ev[:], ps1[:])
                            nc.vector.tensor_tensor(
                                out=run[:], in0=run[:], in1=ev[:], op=Alu.max
                            )
                    res = opool.tile([128, BCH], u8, tag="res")
                    nc.scalar.activation(res[:], run[:], Act.Relu, bias=bias15[:])
                    nc.sync.dma_start(
                        out_t.ap()[:, BCH * bc : BCH * (bc + 1)], res[:]
                    )

            if loop and reps > 1:
                with tc.For_i(0, reps):
                    for _ in range(inner):
                        body()
            else:
                for _ in range(reps):
                    body()
    nc.compile()
    return nc


def _prep_weights(weights):
    """Weight-derived prep (feature permutation, per-core fp8 M, n4),
    cached by a hash of the weight bytes.

    Row layout (slot s of the 1024-padded contraction):
      s = 0: const-1 row carrying each gate's base count
      s in [1, 784]: feature perm[s-1]; NTAIL features chosen to minimize
        n4 are placed at slots [768, 785) (pass 4); slots >= 785 are zero.
    Per output o, its 32 gates are ordered so gates referencing pass-4 slots
    come last; n4 = max over outputs of the count of such gates.
    """
    import hashlib

    import ml_dtypes

    f8np = ml_dtypes.float8_e4m3

    w = np.ascontiguousarray(np.asarray(weights).astype(np.int64)).reshape(
        OUT * OR_T, AND_T
    )
    key = hashlib.sha256(w.tobytes()).hexdigest()
    if _cache.get("wkey") == key:
        return _cache["wprep"]

    v = w.ravel()
    neg = v >= (1 + F)
    f_idx = np.where(neg, v - 1 - F, v - 1)  # feature id, -1 for const
    sgn = np.where(neg, -1.0, 1.0).astype(np.float32)
    sel = v >= 1

    # permute features: pick the NTAIL pass-4 features to minimize
    # n4 = max over outputs of (#gates referencing a pass-4 feature),
    # since that max sets how many tiles need the 4th matmul pass.
    # Greedy build-from-empty with incremental evaluation + swap polish.
    gate_feats = [
        np.unique(f_idx.reshape(-1, AND_T)[g][sel.reshape(-1, AND_T)[g]])
        for g in range(OUT * OR_T)
    ]
    feat_gates = [[] for _ in range(F)]
    for g, fs in enumerate(gate_feats):
        for f in fs:
            feat_gates[f].append(g)
    feat_gates = [np.array(gs, np.int64) for gs in feat_gates]
    gate_out = np.repeat(np.arange(OUT), OR_T)

    cnt = np.zeros(OUT * OR_T, np.int16)  # tail features per gate
    npo = np.zeros(OUT, np.int64)  # needy gates per output

    def score(npo_arr):
        m = npo_arr.max()
        return (m, (npo_arr == m).sum(), npo_arr.sum())

    def add_delta(f):
        gs = feat_gates[f]
        fresh = gs[cnt[gs] == 0]
        return np.bincount(gate_out[fresh], minlength=OUT)

    def apply_add(f):
        gs = feat_gates[f]
        fresh = gs[cnt[gs] == 0]
        cnt[gs] += 1
        npo[:] += np.bincount(gate_out[fresh], minlength=OUT)

    def apply_remove(f):
        gs = feat_gates[f]
        cnt[gs] -= 1
        gone = gs[cnt[gs] == 0]
        npo[:] -= np.bincount(gate_out[gone], minlength=OUT)

    tail = []
    for _ in range(NTAIL):
        best_f, best_s = None, None
        for f in range(F):
            if f in tail:
                continue
            s = score(npo + add_delta(f))
            if best_s is None or s < best_s:
                best_f, best_s = f, s
        apply_add(best_f)
        tail.append(best_f)
    for _ in range(3):  # swap polish
        improved = False
        for i in range(NTAIL):
            f_out = tail[i]
            apply_remove(f_out)
            cur_s = score(npo + add_delta(f_out))
            best_f, best_s = f_out, cur_s
            for f in range(F):
                if f in tail:
                    continue
                s = score(npo + add_delta(f))
                if s < best_s:
                    best_f, best_s = f, s
            apply_add(best_f)
            tail[i] = best_f
            improved = improved or best_f != f_out
        if not improved:
            break

    tail_arr = np.array(sorted(tail), np.int64)
    head_arr = np.setdiff1d(np.arange(F), tail_arr)
    perm = np.concatenate([head_arr, tail_arr])  # tail at slots [768, 785)
    slot_of_feat = np.empty(F, np.int64)
    slot_of_feat[perm] = 1 + np.arange(F)

    ot_ids = np.repeat(np.arange(OUT * OR_T), AND_T)
    Mt = np.zeros((OUT * OR_T, 1024), np.float32)  # [ot, slot]
    np.add.at(Mt, (ot_ids[sel], slot_of_feat[f_idx[sel]]), sgn[sel])
    base = (v == 0).reshape(-1, AND_T).sum(1) + neg.reshape(-1, AND_T).sum(1)
    Mt[:, 0] = base.astype(np.float32)
    allzero = (w == 0).all(1)
    Mt[allzero] = 0.0

    # order each output's gates: pass-4-referencing gates last
    needy = (Mt[:, 768:NROW] != 0).any(1).reshape(OUT, OR_T)
    n4 = int(needy.sum(1).max())
    t_order = np.argsort(needy, axis=1, kind="stable")  # False first
    Mt = Mt.reshape(OUT, OR_T, 1024)
    Mt = np.take_along_axis(Mt, t_order[:, :, None], axis=1)
    # sanity: tiles below OR_T - n4 must have all-zero pass-4 rows
    assert not (Mt[:, : OR_T - n4, 768:NROW] != 0).any()

    # per-core column order: tile t holds columns (o=0..127, slot t)
    Mt = Mt.reshape(N_CORES, O_LOC, OR_T, 1024).transpose(0, 2, 1, 3)
    Mt = Mt.reshape(N_CORES, OT_LOC, 1024)

    # fp8 encodings, row-major layout: slot = 256k + 128j + p
    Mq = Mt.transpose(2, 0, 1).astype(f8np).view(np.uint8)  # [1024s, nc, 4096]
    Mq = Mq.reshape(NK, 2, 128, N_CORES, OT_LOC).transpose(3, 2, 0, 1, 4)
    mq_cores = [np.ascontiguousarray(Mq[cc]) for cc in range(N_CORES)]

    _cache["wkey"] = key
    _cache["wprep"] = (perm, mq_cores, n4)
    return _cache["wprep"]


def _host_inputs(x, weights):
    """Returns (xq, mq_cores, n4)."""
    import ml_dtypes

    f8np = ml_dtypes.float8_e4m3

    perm, mq_cores, n4 = _prep_weights(weights)
    x = np.ascontiguousarray(np.asarray(x, dtype=np.float32))
    xT = np.zeros((1024, B), np.float32)
    xT[0] = 1.0
    xT[1 : 1 + F] = (x.T[perm] != 0).astype(np.float32)
    xq = xT.astype(f8np).view(np.uint8)
    xq = np.ascontiguousarray(xq.reshape(NK, 2, 128, B).transpose(2, 0, 1, 3))

    return xq, mq_cores, n4


def _assemble(results):
    out = np.zeros((B, OUT), dtype=bool)
    for cc in range(N_CORES):
        r = np.ascontiguousarray(results[cc]["out"]).view(np.uint8)
        out[:, O_LOC * cc : O_LOC * (cc + 1)] = (r != 0).T
    return out


def kernel(x, weights):
    from concourse.bass_utils import run_bass_kernel_spmd

    xq, mq_cores, n4 = _host_inputs(x, weights)
    key = ("nc", n4)
    if key not in _cache:
        _cache[key] = _build(reps=1, n4=n4)
    nc = _cache[key]

    in_maps = [{"xq": xq, "mq": mq_cores[cc]} for cc in range(N_CORES)]
    try:
        res = run_bass_kernel_spmd(nc, in_maps, core_ids=list(range(N_CORES)))
    except Exception:
        # transient device/tunnel errors: retry once on a fresh attempt
        res = run_bass_kernel_spmd(nc, in_maps, core_ids=list(range(N_CORES)))
    return _assemble(res.results)
